# revision 25
# baseline (speedup 1.0000x reference)
"""Causal attention (QKV proj + softmax(QK^T/sqrt(d))V) on 8 TRN2 NeuronCores.

Sharding: data-parallel over batch (B=8, one batch element per core).
Per-core kernel, all matmuls in bf16 (1 cyc/col stream + FWL fast weight
load; f32r streams at the same rate but its 4-byte LDWEIGHTS can't use
FWL and stays exposed):
  phase 0: x^T built as [P, dc, t] bf16. t-slice 0 via PE transposes
           (starts ~1us in, keeps PE warm); t-slices 1-3 via the DMA
           xbar transpose engine (x cast to bf16, staged through DRAM,
           transposed DRAM->SBUF in [512,128] tiles) - zero PE cost.
  phase 1: per t-slice: Q then K projections (weights resident bf16),
           evicted to resident Q^T / K^T (no DRAM roundtrip); then
           V = x @ Wv into a separate resident V tile.
  phase 2: per 512-wide query supertile: S^T = K Q^T (so softmax probs
           are produced directly in the lhsT layout needed by P@V),
           with the diagonal band trimmed to the causal range, exp on
           ACT with fused 1/sqrt(D) scale, 128x128 causal mask on the
           diagonal block only, P@V with interleaved ones-matmul row
           sums on PE, reciprocal normalize, store.

DMA queues: sync(HWDGE) = x loads + half the xbar transposes;
scalar(HWDGE) = bf16-x stores, other half of xbar, output stores;
gpsimd(SWDGE) = all weight loads.
"""

import numpy as np

T = 2048
D = 1024
E = 1024
N_CORES = 8
P = 128
TS = 512  # t-slice / supertile width
SCALE = 1.0 / 32.0  # 1/sqrt(D)

DC = D // P  # 8 d-chunks
EC = E // P  # 8 e-chunks
TB = T // P  # 16 t-blocks of 128
NTS = T // TS  # 4 t-slices of 512
JB = TS // P  # 4 q-blocks per supertile
QB = TB // 4  # pT quarter size in k-blocks
EH = E // TS  # 2 e-halves


def _attention_kernel(ctx, tc, out, xb, wq, wk, wv):
    import concourse.bass as bass
    from concourse import mybir
    from concourse.bass import ts
    from concourse.masks import make_identity

    nc = tc.nc
    f32 = mybir.dt.float32
    f32r = mybir.dt.float32r
    bf16 = mybir.dt.bfloat16
    AF = mybir.ActivationFunctionType

    # ---- left-side SBUF pools ----
    const = ctx.enter_context(tc.tile_pool(name="const", bufs=1))
    ones_f32 = const.tile([P, 2], f32)
    nc.vector.memset(ones_f32[:], 1.0)
    ones_col = const.tile([P, 2], bf16)
    nc.vector.tensor_copy(ones_col[:], ones_f32[:])
    # warm the ACT exp table set at program start (off the critical path)
    exp_warm = const.tile([P, 2], f32)
    nc.scalar.activation(exp_warm[:], ones_f32[:], AF.Exp)
    identity_f32 = const.tile([P, P], f32)
    make_identity(nc, identity_f32[:])
    identity = const.tile([P, P], bf16)
    nc.vector.tensor_copy(identity[:], identity_f32[:])

    kt_pool = ctx.enter_context(tc.tile_pool(name="ktres", bufs=1))
    KT = kt_pool.tile([P, EC, T], bf16)  # K^T[e, t], e = ec*128 + ep
    qt_pool = ctx.enter_context(tc.tile_pool(name="qtres", bufs=1))
    QT = qt_pool.tile([P, EC, T], bf16)  # Q^T[e, t], resident

    # 128x128 causal mask for the diagonal blocks of S^T: keep f >= p
    # (p = key partition, f = query col within the block).
    mask_pool = ctx.enter_context(tc.tile_pool(name="maskp", bufs=1))
    mask_f32 = mask_pool.tile([P, P], f32)
    nc.gpsimd.memset(mask_f32[:], 1.0)
    nc.gpsimd.affine_select(
        out=mask_f32[:],
        in_=mask_f32[:],
        compare_op=mybir.AluOpType.is_ge,
        fill=0.0,
        base=0,
        pattern=[[1, P]],
        channel_multiplier=-1,
    )
    mask128 = mask_pool.tile([P, P], bf16)
    nc.vector.tensor_copy(mask128[:], mask_f32[:])

    # ---- right-side work pools ----
    tc.swap_default_side()
    xv_pool = ctx.enter_context(tc.tile_pool(name="xv", bufs=1))
    xv = xv_pool.tile([P, DC, T], bf16)  # x^T[d, t]: [dp, dc, t]
    v_pool = ctx.enter_context(tc.tile_pool(name="vres", bufs=1))
    V = v_pool.tile([P, TB, E], bf16)  # V[t, e]: [tl, tb, e]
    wres_pool = tc.alloc_tile_pool(name="wres", bufs=16)
    wvh_pool = tc.alloc_tile_pool(name="wvh", bufs=2)
    wstg_pool = tc.alloc_tile_pool(name="wstg", bufs=2)
    xa_pool = tc.alloc_tile_pool(name="xa", bufs=8)
    tc.swap_default_side()

    # ---- PSUM pools for phases 0/1 ----
    ps_tp = tc.alloc_tile_pool(name="ps_tp", bufs=2, space="PSUM")
    ps_proj = tc.alloc_tile_pool(name="ps_proj", bufs=6, space="PSUM")

    # ===== phases 0+1a interleaved: x^T bf16 PE transposes for the =====
    # next t-slice are emitted between the Q/K projection groups of the
    # previous one, so x DMA streams behind a full projection window.
    def transpose_tslice(tsl):
        for tb in range(4 * tsl, 4 * tsl + 4):
            transpose_tblock(tb)

    def transpose_tblock(tb):
        if True:
            xa = xa_pool.tile([P, D], bf16, tag="xa", name=f"xa_{tb}")
            if tb < 1:
                # per-dc loads split across two queues so the first
                # transposes start ~8x earlier
                for dc in range(DC):
                    deng = nc.sync if dc % 2 == 0 else nc.gpsimd
                    deng.dma_start(xa[:, ts(dc, P)], xb[ts(tb, P), ts(dc, P)])
            else:
                nc.sync.dma_start(xa[:, 0 : D // 2], xb[ts(tb, P), 0 : D // 2])
                nc.sync.dma_start(xa[:, D // 2 : D], xb[ts(tb, P), D // 2 : D])
            for dh in range(2):
                pt = ps_tp.tile([P, 4, P], bf16)
                for i in range(4):
                    nc.tensor.transpose(
                        pt[:, i, :], xa[:, ts(4 * dh + i, P)], identity[:]
                    )
                dst = xv[:, 4 * dh : 4 * dh + 4, ts(tb, P)]
                if dh == 0:
                    nc.vector.tensor_copy(dst, pt[:])
                else:
                    nc.scalar.copy(dst, pt[:])

    def project_tslice(tsl, tp_next):
        grp = 0
        for wr_lst, dst in ((wr_q, QT), (wr_k, KT)):
            for eb in range(EC):
                pp = ps_proj.tile([P, TS], f32)
                for dc in range(DC):
                    nc.tensor.matmul(
                        pp[:],
                        wr_lst[eb][:, dc, :],
                        xv[:, dc, ts(tsl, TS)],
                        start=(dc == 0),
                        stop=(dc == DC - 1),
                    )
                if eb % 2 == 0:
                    nc.vector.tensor_copy(dst[:, eb, ts(tsl, TS)], pp[:])
                else:
                    nc.scalar.copy(dst[:, eb, ts(tsl, TS)], pp[:])
                # one transpose t-block of the next t-slice every 4 groups
                if grp % 4 == 1 and tp_next is not None:
                    transpose_tblock(4 * tp_next + grp // 4)
                grp += 1

    transpose_tslice(0)
    # ---- resident bf16 weight tiles: Wq/Wk as 8 e-blocks each ----
    # DMAs on the gpsimd SWDGE queue; casts chase the DMAs on DVE/ACT.
    wq_view = wq.rearrange("(dc dp) e -> dp dc e", dp=P)
    wk_view = wk.rearrange("(dc dp) e -> dp dc e", dp=P)
    wv_view = wv.rearrange("(dc dp) e -> dp dc e", dp=P)
    wr_q, wr_k = [], []
    for w_view, lst, nm in ((wq_view, wr_q, "q"), (wk_view, wr_k, "k")):
        for eb in range(EC):
            wstg = wstg_pool.tile(
                [P, DC, P], f32, tag="wstg", name=f"ws{nm}_{eb}"
            )
            nc.gpsimd.dma_start(wstg[:], w_view[:, :, ts(eb, P)])
            wr = wres_pool.tile([P, DC, P], bf16, tag="wres", name=f"wr{nm}_{eb}")
            if eb % 2 == 0:
                nc.vector.tensor_copy(wr[:], wstg[:])
            else:
                nc.scalar.copy(wr[:], wstg[:])
            lst.append(wr)

    for tsl in range(NTS):
        project_tslice(tsl, tsl + 1 if tsl + 1 < NTS else None)

    # ===== phase 1b: V = x @ Wv into the resident V tile =====
    wvhs = []
    for eh in range(EH):
        wvh = wvh_pool.tile([P, DC, TS], bf16, tag="wvh", name=f"wvh_{eh}")
        for qt in range(4):
            wstg = wstg_pool.tile(
                [P, DC, P], f32, tag="wstg", name=f"wsv_{eh}_{qt}"
            )
            nc.gpsimd.dma_start(
                wstg[:], wv_view[:, :, ts(4 * eh + qt, P)]
            )
            if qt % 2 == 0:
                nc.vector.tensor_copy(wvh[:, :, ts(qt, P)], wstg[:])
            else:
                nc.scalar.copy(wvh[:, :, ts(qt, P)], wstg[:])
        wvhs.append(wvh)
    for tb in range(TB):
        for eh in range(EH):
            pp = ps_proj.tile([P, TS], f32)
            for dc in range(DC):
                nc.tensor.matmul(
                    pp[:],
                    xv[:, dc, ts(tb, P)],
                    wvhs[eh][:, dc, :],
                    start=(dc == 0),
                    stop=(dc == DC - 1),
                )
            if eh == 0:
                nc.vector.tensor_copy(V[:, tb, ts(eh, TS)], pp[:])
            else:
                nc.scalar.copy(V[:, tb, ts(eh, TS)], pp[:])

    xa_pool.release()
    wstg_pool.release()
    wvh_pool.release()
    wres_pool.release()
    ps_proj.release()
    ps_tp.release()

    # ================= phase 2: attention =================
    ps_s = tc.alloc_tile_pool(name="ps_s", bufs=4, space="PSUM")
    ps_o = tc.alloc_tile_pool(name="ps_o", bufs=2, space="PSUM")
    ps_sum = tc.alloc_tile_pool(name="ps_sum", bufs=2, space="PSUM")

    tc.swap_default_side()
    pt_pool = ctx.enter_context(tc.tile_pool(name="pt", bufs=5))
    rs_pool = ctx.enter_context(tc.tile_pool(name="rs", bufs=8))
    ostg = ctx.enter_context(tc.tile_pool(name="ostg", bufs=3))
    tc.swap_default_side()

    for sup in range(NTS):
        nkb = JB * sup + JB  # key blocks 0..nkb-1
        pt_parts = [
            pt_pool.tile([P, QB, TS], bf16, tag="pt", name=f"ptp_{sup}_0")
        ]

        # --- S^T blocks + exp + causal mask (diagonal band trimmed) ---
        for k in range(nkb):
            j = k - JB * sup  # >= 0 on the diagonal band
            lo = max(0, j) * P  # first causal query col in this supertile
            ssp = ps_s.tile([P, TS], f32)
            for ec in range(EC):
                nc.tensor.matmul(
                    ssp[:, lo:TS],
                    KT[:, ec, ts(k, P)],
                    QT[:, ec, sup * TS + lo : (sup + 1) * TS],
                    start=(ec == 0),
                    stop=(ec == EC - 1),
                )
            if k // QB >= len(pt_parts):
                pt_parts.append(
                    pt_pool.tile(
                        [P, QB, TS], bf16, tag="pt",
                        name=f"ptp_{sup}_{k // QB}",
                    )
                )
            pk = pt_parts[k // QB][:, k % QB, :]
            nc.scalar.activation(pk[:, lo:TS], ssp[:, lo:TS], AF.Exp, scale=SCALE)
            if j >= 0:
                nc.vector.tensor_mul(
                    pk[:, lo : lo + P], pk[:, lo : lo + P], mask128[:]
                )

        # --- P @ V (+ row sums interleaved in eh=0), normalize, store ---
        rss = {}
        for eh in range(EH):
            for jq in range(JB):
                qb = JB * sup + jq
                nk = qb + 1
                po = ps_o.tile([P, TS], f32)
                if eh == 0:
                    pos = ps_sum.tile([P, 2], f32)
                for k in range(nk):
                    lhsT = pt_parts[k // QB][:, k % QB, ts(jq, P)]
                    nc.tensor.matmul(
                        po[:],
                        lhsT,
                        V[:, k, ts(eh, TS)],
                        start=(k == 0),
                        stop=(k == nk - 1),
                    )
                    if eh == 0:
                        nc.tensor.matmul(
                            pos[:],
                            lhsT,
                            ones_col[:],
                            start=(k == 0),
                            stop=(k == nk - 1),
                        )
                if eh == 0:
                    rs = rs_pool.tile(
                        [P, 1], f32, tag="rs", name=f"rs_{sup}_{jq}"
                    )
                    nc.vector.reciprocal(rs[:], pos[:, 0:1])
                    rss[jq] = rs
                ost = ostg.tile([P, TS], f32, tag="ostage")
                nc.scalar.activation(
                    ost[:], po[:], AF.Copy, scale=rss[jq][:]
                )
                seng = nc.scalar if jq % 2 == 0 else nc.sync
                seng.dma_start(out[ts(qb, P), ts(eh, TS)], ost[:])

    ps_sum.release()
    ps_o.release()
    ps_s.release()


def build_program():
    from contextlib import ExitStack

    import concourse.bacc as bacc
    import concourse.tile as tile
    from concourse import mybir

    nc = bacc.Bacc("TRN2", target_bir_lowering=False, debug=False)
    f32 = mybir.dt.float32
    xb = nc.dram_tensor("xb16", [T, D], mybir.dt.bfloat16, kind="ExternalInput").ap()
    wq = nc.dram_tensor("Wq", [D, E], f32, kind="ExternalInput").ap()
    wk = nc.dram_tensor("Wk", [D, E], f32, kind="ExternalInput").ap()
    wv = nc.dram_tensor("Wv", [D, E], f32, kind="ExternalInput").ap()
    out = nc.dram_tensor("out", [T, E], f32, kind="ExternalOutput").ap()

    with tile.TileContext(nc) as tc:
        with ExitStack() as ctx:
            _attention_kernel(ctx, tc, out, xb, wq, wk, wv)
    nc.compile()
    return nc


def kernel(x, Wq, Wk, Wv, _trace=False):
    from concourse.bass_utils import run_bass_kernel_spmd

    x = np.ascontiguousarray(np.asarray(x), dtype=np.float32)
    Wq = np.ascontiguousarray(np.asarray(Wq), dtype=np.float32)
    Wk = np.ascontiguousarray(np.asarray(Wk), dtype=np.float32)
    Wv = np.ascontiguousarray(np.asarray(Wv), dtype=np.float32)
    assert x.shape == (N_CORES, T, D), x.shape

    nc = build_program()
    import ml_dtypes

    xb16 = np.ascontiguousarray(x.astype(ml_dtypes.bfloat16))
    in_maps = [
        {"xb16": xb16[b], "Wq": Wq, "Wk": Wk, "Wv": Wv}
        for b in range(N_CORES)
    ]
    last_err = None
    for attempt in range(3):
        try:
            res = run_bass_kernel_spmd(
                nc, in_maps, core_ids=list(range(N_CORES)), trace=_trace
            )
            break
        except Exception as e:  # transient device wedge: retry
            last_err = e
            import time

            time.sleep(5.0 * (attempt + 1))
    else:
        raise last_err
    out = np.stack([res.results[b]["out"] for b in range(N_CORES)], axis=0)
    if _trace:
        kernel.last_results = res
    return out


kernel.last_results = None


# revision 26
# speedup vs baseline: 1.0101x; 1.0101x over previous
"""Causal attention (QKV proj + softmax(QK^T/sqrt(d))V) on 8 TRN2 NeuronCores.

Sharding: data-parallel over batch (B=8, one batch element per core).
Per-core kernel, all matmuls in bf16 (1 cyc/col stream + FWL fast weight
load; f32r streams at the same rate but its 4-byte LDWEIGHTS can't use
FWL and stays exposed):
  phase 0: x^T built as [P, dc, t] bf16. t-slice 0 via PE transposes
           (starts ~1us in, keeps PE warm); t-slices 1-3 via the DMA
           xbar transpose engine (x cast to bf16, staged through DRAM,
           transposed DRAM->SBUF in [512,128] tiles) - zero PE cost.
  phase 1: per t-slice: Q then K projections (weights resident bf16),
           evicted to resident Q^T / K^T (no DRAM roundtrip); then
           V = x @ Wv into a separate resident V tile.
  phase 2: per 512-wide query supertile: S^T = K Q^T (so softmax probs
           are produced directly in the lhsT layout needed by P@V),
           with the diagonal band trimmed to the causal range, exp on
           ACT with fused 1/sqrt(D) scale, 128x128 causal mask on the
           diagonal block only, P@V with interleaved ones-matmul row
           sums on PE, reciprocal normalize, store.

DMA queues: sync(HWDGE) = x loads + half the xbar transposes;
scalar(HWDGE) = bf16-x stores, other half of xbar, output stores;
gpsimd(SWDGE) = all weight loads.
"""

import numpy as np

T = 2048
D = 1024
E = 1024
N_CORES = 8
P = 128
TS = 512  # t-slice / supertile width
SCALE = 1.0 / 32.0  # 1/sqrt(D)

DC = D // P  # 8 d-chunks
EC = E // P  # 8 e-chunks
TB = T // P  # 16 t-blocks of 128
NTS = T // TS  # 4 t-slices of 512
JB = TS // P  # 4 q-blocks per supertile
QB = TB // 4  # pT quarter size in k-blocks
EH = E // TS  # 2 e-halves


def _attention_kernel(ctx, tc, out, xb, wq, wk, wv):
    import concourse.bass as bass
    from concourse import mybir
    from concourse.bass import ts
    from concourse.masks import make_identity

    nc = tc.nc
    f32 = mybir.dt.float32
    f32r = mybir.dt.float32r
    bf16 = mybir.dt.bfloat16
    AF = mybir.ActivationFunctionType

    # ---- left-side SBUF pools ----
    const = ctx.enter_context(tc.tile_pool(name="const", bufs=1))
    ones_f32 = const.tile([P, 2], f32)
    nc.vector.memset(ones_f32[:], 1.0)
    ones_col = const.tile([P, 2], bf16)
    nc.vector.tensor_copy(ones_col[:], ones_f32[:])
    # warm the ACT exp table set at program start (off the critical path)
    exp_warm = const.tile([P, 2], f32)
    nc.scalar.activation(exp_warm[:], ones_f32[:], AF.Exp)
    identity_f32 = const.tile([P, P], f32)
    make_identity(nc, identity_f32[:])
    identity = const.tile([P, P], bf16)
    nc.vector.tensor_copy(identity[:], identity_f32[:])

    kt_pool = ctx.enter_context(tc.tile_pool(name="ktres", bufs=1))
    KT = kt_pool.tile([P, EC, T], bf16)  # K^T[e, t], e = ec*128 + ep
    qt_pool = ctx.enter_context(tc.tile_pool(name="qtres", bufs=1))
    QT = qt_pool.tile([P, EC, T], bf16)  # Q^T[e, t], resident

    # 128x128 causal mask for the diagonal blocks of S^T: keep f >= p
    # (p = key partition, f = query col within the block).
    mask_pool = ctx.enter_context(tc.tile_pool(name="maskp", bufs=1))
    mask_f32 = mask_pool.tile([P, P], f32)
    nc.gpsimd.memset(mask_f32[:], 1.0)
    nc.gpsimd.affine_select(
        out=mask_f32[:],
        in_=mask_f32[:],
        compare_op=mybir.AluOpType.is_ge,
        fill=0.0,
        base=0,
        pattern=[[1, P]],
        channel_multiplier=-1,
    )
    mask128 = mask_pool.tile([P, P], bf16)
    nc.vector.tensor_copy(mask128[:], mask_f32[:])

    # ---- right-side work pools ----
    tc.swap_default_side()
    xv_pool = ctx.enter_context(tc.tile_pool(name="xv", bufs=1))
    xv = xv_pool.tile([P, DC, T], bf16)  # x^T[d, t]: [dp, dc, t]
    v_pool = ctx.enter_context(tc.tile_pool(name="vres", bufs=1))
    V = v_pool.tile([P, TB, E], bf16)  # V[t, e]: [tl, tb, e]
    wres_pool = tc.alloc_tile_pool(name="wres", bufs=16)
    wvh_pool = tc.alloc_tile_pool(name="wvh", bufs=2)
    wstg_pool = tc.alloc_tile_pool(name="wstg", bufs=2)
    xa_pool = tc.alloc_tile_pool(name="xa", bufs=8)
    tc.swap_default_side()

    # ---- PSUM pools for phases 0/1 ----
    ps_tp = tc.alloc_tile_pool(name="ps_tp", bufs=2, space="PSUM")
    ps_proj = tc.alloc_tile_pool(name="ps_proj", bufs=6, space="PSUM")

    # ---- resident bf16 weight tiles: Wq/Wk as 8 e-blocks each ----
    # DMAs on the gpsimd SWDGE queue; casts chase the DMAs on DVE/ACT.
    wq_view = wq.rearrange("(dc dp) e -> dp dc e", dp=P)
    wk_view = wk.rearrange("(dc dp) e -> dp dc e", dp=P)
    wv_view = wv.rearrange("(dc dp) e -> dp dc e", dp=P)
    wr_q, wr_k = [], []
    for w_view, lst, nm in ((wq_view, wr_q, "q"), (wk_view, wr_k, "k")):
        for eb in range(EC):
            wstg = wstg_pool.tile(
                [P, DC, P], f32, tag="wstg", name=f"ws{nm}_{eb}"
            )
            nc.gpsimd.dma_start(wstg[:], w_view[:, :, ts(eb, P)])
            wr = wres_pool.tile([P, DC, P], bf16, tag="wres", name=f"wr{nm}_{eb}")
            if eb % 2 == 0:
                nc.vector.tensor_copy(wr[:], wstg[:])
            else:
                nc.scalar.copy(wr[:], wstg[:])
            lst.append(wr)


    # ===== phases 0+1a interleaved: x^T bf16 PE transposes for the =====
    # next t-slice are emitted between the Q/K projection groups of the
    # previous one, so x DMA streams behind a full projection window.
    def transpose_tslice(tsl):
        for tb in range(4 * tsl, 4 * tsl + 4):
            transpose_tblock(tb)

    def transpose_tblock(tb):
        if True:
            xa = xa_pool.tile([P, D], bf16, tag="xa", name=f"xa_{tb}")
            if tb < 1:
                # per-dc loads split across two queues so the first
                # transposes start ~8x earlier
                for dc in range(DC):
                    deng = nc.sync if dc % 2 == 0 else nc.gpsimd
                    deng.dma_start(xa[:, ts(dc, P)], xb[ts(tb, P), ts(dc, P)])
            else:
                nc.sync.dma_start(xa[:, 0 : D // 2], xb[ts(tb, P), 0 : D // 2])
                nc.sync.dma_start(xa[:, D // 2 : D], xb[ts(tb, P), D // 2 : D])
            for dh in range(2):
                pt = ps_tp.tile([P, 4, P], bf16)
                for i in range(4):
                    nc.tensor.transpose(
                        pt[:, i, :], xa[:, ts(4 * dh + i, P)], identity[:]
                    )
                dst = xv[:, 4 * dh : 4 * dh + 4, ts(tb, P)]
                if dh == 0:
                    nc.vector.tensor_copy(dst, pt[:])
                else:
                    nc.scalar.copy(dst, pt[:])

    def project_tslice(tsl, tp_next):
        grp = 0
        for wr_lst, dst in ((wr_q, QT), (wr_k, KT)):
            for eb in range(EC):
                pp = ps_proj.tile([P, TS], f32)
                for dc in range(DC):
                    nc.tensor.matmul(
                        pp[:],
                        wr_lst[eb][:, dc, :],
                        xv[:, dc, ts(tsl, TS)],
                        start=(dc == 0),
                        stop=(dc == DC - 1),
                    )
                if eb % 2 == 0:
                    nc.vector.tensor_copy(dst[:, eb, ts(tsl, TS)], pp[:])
                else:
                    nc.scalar.copy(dst[:, eb, ts(tsl, TS)], pp[:])
                # one transpose t-block of the next t-slice every 4 groups
                if grp % 4 == 1 and tp_next is not None:
                    transpose_tblock(4 * tp_next + grp // 4)
                grp += 1

    transpose_tslice(0)
    for tsl in range(NTS):
        project_tslice(tsl, tsl + 1 if tsl + 1 < NTS else None)

    # ===== phase 1b: V = x @ Wv into the resident V tile =====
    wvhs = []
    for eh in range(EH):
        wvh = wvh_pool.tile([P, DC, TS], bf16, tag="wvh", name=f"wvh_{eh}")
        for qt in range(4):
            wstg = wstg_pool.tile(
                [P, DC, P], f32, tag="wstg", name=f"wsv_{eh}_{qt}"
            )
            nc.gpsimd.dma_start(
                wstg[:], wv_view[:, :, ts(4 * eh + qt, P)]
            )
            if qt % 2 == 0:
                nc.vector.tensor_copy(wvh[:, :, ts(qt, P)], wstg[:])
            else:
                nc.scalar.copy(wvh[:, :, ts(qt, P)], wstg[:])
        wvhs.append(wvh)
    for tb in range(TB):
        for eh in range(EH):
            pp = ps_proj.tile([P, TS], f32)
            for dc in range(DC):
                nc.tensor.matmul(
                    pp[:],
                    xv[:, dc, ts(tb, P)],
                    wvhs[eh][:, dc, :],
                    start=(dc == 0),
                    stop=(dc == DC - 1),
                )
            if eh == 0:
                nc.vector.tensor_copy(V[:, tb, ts(eh, TS)], pp[:])
            else:
                nc.scalar.copy(V[:, tb, ts(eh, TS)], pp[:])

    xa_pool.release()
    wstg_pool.release()
    wvh_pool.release()
    wres_pool.release()
    ps_proj.release()
    ps_tp.release()

    # ================= phase 2: attention =================
    ps_s = tc.alloc_tile_pool(name="ps_s", bufs=4, space="PSUM")
    ps_o = tc.alloc_tile_pool(name="ps_o", bufs=2, space="PSUM")
    ps_sum = tc.alloc_tile_pool(name="ps_sum", bufs=2, space="PSUM")

    tc.swap_default_side()
    pt_pool = ctx.enter_context(tc.tile_pool(name="pt", bufs=5))
    rs_pool = ctx.enter_context(tc.tile_pool(name="rs", bufs=8))
    ostg = ctx.enter_context(tc.tile_pool(name="ostg", bufs=3))
    tc.swap_default_side()

    for sup in range(NTS):
        nkb = JB * sup + JB  # key blocks 0..nkb-1
        pt_parts = [
            pt_pool.tile([P, QB, TS], bf16, tag="pt", name=f"ptp_{sup}_0")
        ]

        # --- S^T blocks + exp + causal mask (diagonal band trimmed) ---
        for k in range(nkb):
            j = k - JB * sup  # >= 0 on the diagonal band
            lo = max(0, j) * P  # first causal query col in this supertile
            ssp = ps_s.tile([P, TS], f32)
            for ec in range(EC):
                nc.tensor.matmul(
                    ssp[:, lo:TS],
                    KT[:, ec, ts(k, P)],
                    QT[:, ec, sup * TS + lo : (sup + 1) * TS],
                    start=(ec == 0),
                    stop=(ec == EC - 1),
                )
            if k // QB >= len(pt_parts):
                pt_parts.append(
                    pt_pool.tile(
                        [P, QB, TS], bf16, tag="pt",
                        name=f"ptp_{sup}_{k // QB}",
                    )
                )
            pk = pt_parts[k // QB][:, k % QB, :]
            nc.scalar.activation(pk[:, lo:TS], ssp[:, lo:TS], AF.Exp, scale=SCALE)
            if j >= 0:
                nc.vector.tensor_mul(
                    pk[:, lo : lo + P], pk[:, lo : lo + P], mask128[:]
                )

        # --- P @ V (+ row sums interleaved in eh=0), normalize, store ---
        rss = {}
        for eh in range(EH):
            for jq in range(JB):
                qb = JB * sup + jq
                nk = qb + 1
                po = ps_o.tile([P, TS], f32)
                if eh == 0:
                    pos = ps_sum.tile([P, 2], f32)
                for k in range(nk):
                    lhsT = pt_parts[k // QB][:, k % QB, ts(jq, P)]
                    nc.tensor.matmul(
                        po[:],
                        lhsT,
                        V[:, k, ts(eh, TS)],
                        start=(k == 0),
                        stop=(k == nk - 1),
                    )
                    if eh == 0:
                        nc.tensor.matmul(
                            pos[:],
                            lhsT,
                            ones_col[:],
                            start=(k == 0),
                            stop=(k == nk - 1),
                        )
                if eh == 0:
                    rs = rs_pool.tile(
                        [P, 1], f32, tag="rs", name=f"rs_{sup}_{jq}"
                    )
                    nc.vector.reciprocal(rs[:], pos[:, 0:1])
                    rss[jq] = rs
                ost = ostg.tile([P, TS], f32, tag="ostage")
                nc.scalar.activation(
                    ost[:], po[:], AF.Copy, scale=rss[jq][:]
                )
                seng = nc.scalar if jq % 2 == 0 else nc.sync
                seng.dma_start(out[ts(qb, P), ts(eh, TS)], ost[:])

    ps_sum.release()
    ps_o.release()
    ps_s.release()


def build_program():
    from contextlib import ExitStack

    import concourse.bacc as bacc
    import concourse.tile as tile
    from concourse import mybir

    nc = bacc.Bacc("TRN2", target_bir_lowering=False, debug=False)
    f32 = mybir.dt.float32
    xb = nc.dram_tensor("xb16", [T, D], mybir.dt.bfloat16, kind="ExternalInput").ap()
    wq = nc.dram_tensor("Wq", [D, E], f32, kind="ExternalInput").ap()
    wk = nc.dram_tensor("Wk", [D, E], f32, kind="ExternalInput").ap()
    wv = nc.dram_tensor("Wv", [D, E], f32, kind="ExternalInput").ap()
    out = nc.dram_tensor("out", [T, E], f32, kind="ExternalOutput").ap()

    with tile.TileContext(nc) as tc:
        with ExitStack() as ctx:
            _attention_kernel(ctx, tc, out, xb, wq, wk, wv)
    nc.compile()
    return nc


def kernel(x, Wq, Wk, Wv, _trace=False):
    from concourse.bass_utils import run_bass_kernel_spmd

    x = np.ascontiguousarray(np.asarray(x), dtype=np.float32)
    Wq = np.ascontiguousarray(np.asarray(Wq), dtype=np.float32)
    Wk = np.ascontiguousarray(np.asarray(Wk), dtype=np.float32)
    Wv = np.ascontiguousarray(np.asarray(Wv), dtype=np.float32)
    assert x.shape == (N_CORES, T, D), x.shape

    nc = build_program()
    import ml_dtypes

    xb16 = np.ascontiguousarray(x.astype(ml_dtypes.bfloat16))
    in_maps = [
        {"xb16": xb16[b], "Wq": Wq, "Wk": Wk, "Wv": Wv}
        for b in range(N_CORES)
    ]
    last_err = None
    for attempt in range(3):
        try:
            res = run_bass_kernel_spmd(
                nc, in_maps, core_ids=list(range(N_CORES)), trace=_trace
            )
            break
        except Exception as e:  # transient device wedge: retry
            last_err = e
            import time

            time.sleep(5.0 * (attempt + 1))
    else:
        raise last_err
    out = np.stack([res.results[b]["out"] for b in range(N_CORES)], axis=0)
    if _trace:
        kernel.last_results = res
    return out


kernel.last_results = None


# revision 27
# speedup vs baseline: 1.1982x; 1.1862x over previous
"""Causal attention (QKV proj + softmax(QK^T/sqrt(d))V) on 8 TRN2 NeuronCores.

Sharding: data-parallel over batch (B=8, one batch element per core).
Per-core kernel, all matmuls in bf16 (1 cyc/col stream + FWL fast weight
load; f32r streams at the same rate but its 4-byte LDWEIGHTS can't use
FWL and stays exposed):
  phase 0: x^T built as [P, dc, t] bf16. t-slice 0 via PE transposes
           (starts ~1us in, keeps PE warm); t-slices 1-3 via the DMA
           xbar transpose engine (x cast to bf16, staged through DRAM,
           transposed DRAM->SBUF in [512,128] tiles) - zero PE cost.
  phase 1: per t-slice: Q then K projections (weights resident bf16),
           evicted to resident Q^T / K^T (no DRAM roundtrip); then
           V = x @ Wv into a separate resident V tile.
  phase 2: per 512-wide query supertile: S^T = K Q^T (so softmax probs
           are produced directly in the lhsT layout needed by P@V),
           with the diagonal band trimmed to the causal range, exp on
           ACT with fused 1/sqrt(D) scale, 128x128 causal mask on the
           diagonal block only, P@V with interleaved ones-matmul row
           sums on PE, reciprocal normalize, store.

DMA queues: sync(HWDGE) = x loads + half the xbar transposes;
scalar(HWDGE) = bf16-x stores, other half of xbar, output stores;
gpsimd(SWDGE) = all weight loads.
"""

import numpy as np

T = 2048
D = 1024
E = 1024
N_CORES = 8
P = 128
TS = 512  # t-slice / supertile width
SCALE = 1.0 / 32.0  # 1/sqrt(D)

DC = D // P  # 8 d-chunks
EC = E // P  # 8 e-chunks
TB = T // P  # 16 t-blocks of 128
NTS = T // TS  # 4 t-slices of 512
JB = TS // P  # 4 q-blocks per supertile
QB = TB // 4  # pT quarter size in k-blocks
EH = E // TS  # 2 e-halves


def _attention_kernel(ctx, tc, out, xb, wq, wk, wv):
    import concourse.bass as bass
    from concourse import mybir
    from concourse.bass import ts
    from concourse.masks import make_identity

    nc = tc.nc
    f32 = mybir.dt.float32
    f32r = mybir.dt.float32r
    bf16 = mybir.dt.bfloat16
    AF = mybir.ActivationFunctionType

    # ---- left-side SBUF pools ----
    const = ctx.enter_context(tc.tile_pool(name="const", bufs=1))
    ones_f32 = const.tile([P, 2], f32)
    nc.vector.memset(ones_f32[:], 1.0)
    ones_col = const.tile([P, 2], bf16)
    nc.vector.tensor_copy(ones_col[:], ones_f32[:])
    # warm the ACT exp table set at program start (off the critical path)
    exp_warm = const.tile([P, 2], f32)
    nc.scalar.activation(exp_warm[:], ones_f32[:], AF.Exp)
    identity_f32 = const.tile([P, P], f32)
    make_identity(nc, identity_f32[:])
    identity = const.tile([P, P], bf16)
    nc.vector.tensor_copy(identity[:], identity_f32[:])

    kt_pool = ctx.enter_context(tc.tile_pool(name="ktres", bufs=1))
    KT = kt_pool.tile([P, EC, T], bf16)  # K^T[e, t], e = ec*128 + ep
    qt_pool = ctx.enter_context(tc.tile_pool(name="qtres", bufs=1))
    QT = qt_pool.tile([P, EC, T], bf16)  # Q^T[e, t], resident

    # 128x128 causal mask for the diagonal blocks of S^T: keep f >= p
    # (p = key partition, f = query col within the block).
    mask_pool = ctx.enter_context(tc.tile_pool(name="maskp", bufs=1))
    mask_f32 = mask_pool.tile([P, P], f32)
    nc.gpsimd.memset(mask_f32[:], 1.0)
    nc.gpsimd.affine_select(
        out=mask_f32[:],
        in_=mask_f32[:],
        compare_op=mybir.AluOpType.is_ge,
        fill=0.0,
        base=0,
        pattern=[[1, P]],
        channel_multiplier=-1,
    )
    mask128 = mask_pool.tile([P, P], bf16)
    nc.vector.tensor_copy(mask128[:], mask_f32[:])

    # ---- right-side work pools ----
    tc.swap_default_side()
    xv_pool = ctx.enter_context(tc.tile_pool(name="xv", bufs=1))
    xv = xv_pool.tile([P, DC, T], bf16)  # x^T[d, t]: [dp, dc, t]
    v_pool = ctx.enter_context(tc.tile_pool(name="vres", bufs=1))
    V = v_pool.tile([P, TB, E], bf16)  # V[t, e]: [tl, tb, e]
    wres_pool = tc.alloc_tile_pool(name="wres", bufs=16)
    wvh_pool = tc.alloc_tile_pool(name="wvh", bufs=2)
    wstg_pool = tc.alloc_tile_pool(name="wstg", bufs=2)
    xa_pool = tc.alloc_tile_pool(name="xa", bufs=8)
    tc.swap_default_side()

    # ---- PSUM pools for phases 0/1 ----
    ps_tp = tc.alloc_tile_pool(name="ps_tp", bufs=2, space="PSUM")
    ps_proj = tc.alloc_tile_pool(name="ps_proj", bufs=6, space="PSUM")

    # ---- resident bf16 weight tiles: Wq/Wk as 8 e-blocks each ----
    # DMAs on the gpsimd SWDGE queue; casts chase the DMAs on DVE/ACT.
    wq_view = wq.rearrange("(dc dp) e -> dp dc e", dp=P)
    wk_view = wk.rearrange("(dc dp) e -> dp dc e", dp=P)
    wv_view = wv.rearrange("(dc dp) e -> dp dc e", dp=P)
    wr_q, wr_k = [], []
    for w_view, lst, nm in ((wq_view, wr_q, "q"), (wk_view, wr_k, "k")):
        for eb in range(EC):
            wstg = wstg_pool.tile(
                [P, DC, P], f32, tag="wstg", name=f"ws{nm}_{eb}"
            )
            nc.gpsimd.dma_start(wstg[:], w_view[:, :, ts(eb, P)])
            wr = wres_pool.tile([P, DC, P], bf16, tag="wres", name=f"wr{nm}_{eb}")
            if eb % 2 == 0:
                nc.vector.tensor_copy(wr[:], wstg[:])
            else:
                nc.scalar.copy(wr[:], wstg[:])
            lst.append(wr)


    # ===== phases 0+1a interleaved: x^T bf16 PE transposes for the =====
    # next t-slice are emitted between the Q/K projection groups of the
    # previous one, so x DMA streams behind a full projection window.
    def transpose_tslice(tsl):
        for tb in range(4 * tsl, 4 * tsl + 4):
            transpose_tblock(tb)

    def transpose_tblock(tb):
        if True:
            xa = xa_pool.tile([P, D], bf16, tag="xa", name=f"xa_{tb}")
            if tb < 1:
                # per-dc loads so the first transposes start ~4x earlier
                for dc in range(DC):
                    nc.sync.dma_start(xa[:, ts(dc, P)], xb[ts(tb, P), ts(dc, P)])
            else:
                nc.sync.dma_start(xa[:, 0 : D // 2], xb[ts(tb, P), 0 : D // 2])
                nc.sync.dma_start(xa[:, D // 2 : D], xb[ts(tb, P), D // 2 : D])
            for dh in range(2):
                pt = ps_tp.tile([P, 4, P], bf16)
                for i in range(4):
                    nc.tensor.transpose(
                        pt[:, i, :], xa[:, ts(4 * dh + i, P)], identity[:]
                    )
                dst = xv[:, 4 * dh : 4 * dh + 4, ts(tb, P)]
                if dh == 0:
                    nc.vector.tensor_copy(dst, pt[:])
                else:
                    nc.scalar.copy(dst, pt[:])

    def project_tslice(tsl, tp_next):
        grp = 0
        for wr_lst, dst in ((wr_q, QT), (wr_k, KT)):
            for eb in range(EC):
                pp = ps_proj.tile([P, TS], f32)
                for dc in range(DC):
                    nc.tensor.matmul(
                        pp[:],
                        wr_lst[eb][:, dc, :],
                        xv[:, dc, ts(tsl, TS)],
                        start=(dc == 0),
                        stop=(dc == DC - 1),
                    )
                if eb % 2 == 0:
                    nc.vector.tensor_copy(dst[:, eb, ts(tsl, TS)], pp[:])
                else:
                    nc.scalar.copy(dst[:, eb, ts(tsl, TS)], pp[:])
                # one transpose t-block of the next t-slice every 4 groups
                if grp % 4 == 1 and tp_next is not None:
                    transpose_tblock(4 * tp_next + grp // 4)
                grp += 1

    transpose_tslice(0)
    for tsl in range(NTS):
        project_tslice(tsl, tsl + 1 if tsl + 1 < NTS else None)

    # ===== phase 1b: V = x @ Wv into the resident V tile =====
    wvhs = []
    for eh in range(EH):
        wvh = wvh_pool.tile([P, DC, TS], bf16, tag="wvh", name=f"wvh_{eh}")
        for qt in range(4):
            wstg = wstg_pool.tile(
                [P, DC, P], f32, tag="wstg", name=f"wsv_{eh}_{qt}"
            )
            nc.gpsimd.dma_start(
                wstg[:], wv_view[:, :, ts(4 * eh + qt, P)]
            )
            if qt % 2 == 0:
                nc.vector.tensor_copy(wvh[:, :, ts(qt, P)], wstg[:])
            else:
                nc.scalar.copy(wvh[:, :, ts(qt, P)], wstg[:])
        wvhs.append(wvh)
    for tb in range(TB):
        for eh in range(EH):
            pp = ps_proj.tile([P, TS], f32)
            for dc in range(DC):
                nc.tensor.matmul(
                    pp[:],
                    xv[:, dc, ts(tb, P)],
                    wvhs[eh][:, dc, :],
                    start=(dc == 0),
                    stop=(dc == DC - 1),
                )
            if eh == 0:
                nc.vector.tensor_copy(V[:, tb, ts(eh, TS)], pp[:])
            else:
                nc.scalar.copy(V[:, tb, ts(eh, TS)], pp[:])

    xa_pool.release()
    wstg_pool.release()
    wvh_pool.release()
    wres_pool.release()
    ps_proj.release()
    ps_tp.release()

    # ================= phase 2: attention =================
    ps_s = tc.alloc_tile_pool(name="ps_s", bufs=4, space="PSUM")
    ps_o = tc.alloc_tile_pool(name="ps_o", bufs=2, space="PSUM")
    ps_sum = tc.alloc_tile_pool(name="ps_sum", bufs=2, space="PSUM")

    tc.swap_default_side()
    pt_pool = ctx.enter_context(tc.tile_pool(name="pt", bufs=5))
    rs_pool = ctx.enter_context(tc.tile_pool(name="rs", bufs=8))
    ostg = ctx.enter_context(tc.tile_pool(name="ostg", bufs=3))
    tc.swap_default_side()

    for sup in range(NTS):
        nkb = JB * sup + JB  # key blocks 0..nkb-1
        pt_parts = [
            pt_pool.tile([P, QB, TS], bf16, tag="pt", name=f"ptp_{sup}_0")
        ]

        # --- S^T blocks + exp + causal mask (diagonal band trimmed) ---
        for k in range(nkb):
            j = k - JB * sup  # >= 0 on the diagonal band
            lo = max(0, j) * P  # first causal query col in this supertile
            ssp = ps_s.tile([P, TS], f32)
            for ec in range(EC):
                nc.tensor.matmul(
                    ssp[:, lo:TS],
                    KT[:, ec, ts(k, P)],
                    QT[:, ec, sup * TS + lo : (sup + 1) * TS],
                    start=(ec == 0),
                    stop=(ec == EC - 1),
                )
            if k // QB >= len(pt_parts):
                pt_parts.append(
                    pt_pool.tile(
                        [P, QB, TS], bf16, tag="pt",
                        name=f"ptp_{sup}_{k // QB}",
                    )
                )
            pk = pt_parts[k // QB][:, k % QB, :]
            nc.scalar.activation(pk[:, lo:TS], ssp[:, lo:TS], AF.Exp, scale=SCALE)
            if j >= 0:
                nc.vector.tensor_mul(
                    pk[:, lo : lo + P], pk[:, lo : lo + P], mask128[:]
                )

        # --- P @ V (+ row sums interleaved in eh=0), normalize, store ---
        rss = {}
        for eh in range(EH):
            for jq in range(JB):
                qb = JB * sup + jq
                nk = qb + 1
                po = ps_o.tile([P, TS], f32)
                if eh == 0:
                    pos = ps_sum.tile([P, 2], f32)
                for k in range(nk):
                    lhsT = pt_parts[k // QB][:, k % QB, ts(jq, P)]
                    nc.tensor.matmul(
                        po[:],
                        lhsT,
                        V[:, k, ts(eh, TS)],
                        start=(k == 0),
                        stop=(k == nk - 1),
                    )
                    if eh == 0:
                        nc.tensor.matmul(
                            pos[:],
                            lhsT,
                            ones_col[:],
                            start=(k == 0),
                            stop=(k == nk - 1),
                        )
                if eh == 0:
                    rs = rs_pool.tile(
                        [P, 1], f32, tag="rs", name=f"rs_{sup}_{jq}"
                    )
                    nc.vector.reciprocal(rs[:], pos[:, 0:1])
                    rss[jq] = rs
                ost = ostg.tile([P, TS], f32, tag="ostage")
                nc.scalar.activation(
                    ost[:], po[:], AF.Copy, scale=rss[jq][:]
                )
                seng = nc.scalar if jq % 2 == 0 else nc.sync
                seng.dma_start(out[ts(qb, P), ts(eh, TS)], ost[:])

    ps_sum.release()
    ps_o.release()
    ps_s.release()


def build_program():
    from contextlib import ExitStack

    import concourse.bacc as bacc
    import concourse.tile as tile
    from concourse import mybir

    nc = bacc.Bacc("TRN2", target_bir_lowering=False, debug=False)
    f32 = mybir.dt.float32
    xb = nc.dram_tensor("xb16", [T, D], mybir.dt.bfloat16, kind="ExternalInput").ap()
    wq = nc.dram_tensor("Wq", [D, E], f32, kind="ExternalInput").ap()
    wk = nc.dram_tensor("Wk", [D, E], f32, kind="ExternalInput").ap()
    wv = nc.dram_tensor("Wv", [D, E], f32, kind="ExternalInput").ap()
    out = nc.dram_tensor("out", [T, E], f32, kind="ExternalOutput").ap()

    with tile.TileContext(nc) as tc:
        with ExitStack() as ctx:
            _attention_kernel(ctx, tc, out, xb, wq, wk, wv)
    nc.compile()
    return nc


def kernel(x, Wq, Wk, Wv, _trace=False):
    from concourse.bass_utils import run_bass_kernel_spmd

    x = np.ascontiguousarray(np.asarray(x), dtype=np.float32)
    Wq = np.ascontiguousarray(np.asarray(Wq), dtype=np.float32)
    Wk = np.ascontiguousarray(np.asarray(Wk), dtype=np.float32)
    Wv = np.ascontiguousarray(np.asarray(Wv), dtype=np.float32)
    assert x.shape == (N_CORES, T, D), x.shape

    nc = build_program()
    import ml_dtypes

    xb16 = np.ascontiguousarray(x.astype(ml_dtypes.bfloat16))
    in_maps = [
        {"xb16": xb16[b], "Wq": Wq, "Wk": Wk, "Wv": Wv}
        for b in range(N_CORES)
    ]
    last_err = None
    for attempt in range(3):
        try:
            res = run_bass_kernel_spmd(
                nc, in_maps, core_ids=list(range(N_CORES)), trace=_trace
            )
            break
        except Exception as e:  # transient device wedge: retry
            last_err = e
            import time

            time.sleep(5.0 * (attempt + 1))
    else:
        raise last_err
    out = np.stack([res.results[b]["out"] for b in range(N_CORES)], axis=0)
    if _trace:
        kernel.last_results = res
    return out


kernel.last_results = None
